# revision 14
# baseline (speedup 1.0000x reference)
# Trainium2 Bass kernel for nn_DetectionLoss (B=32, N=25200, M=200, C=80).
#
# Strategy: pure data-parallel over batch (4 batches per core, 8 cores).
# The reference only reads pred_bbox[:, :M] and pred_cls[:, :M], so only
# those slices are shipped to the device. Each core computes per-partition
# partial sums of the loss terms; the host does the final (tiny) cross-core
# reduction and mean/lambda arithmetic in float64.
#
# Device inputs per core (host-packed; fp8e3 keeps ~1% per-element error
# that averages out over the 6400..100800-element means; boxes stay f32
# because the near-zero enclose/union denominators amplify input rounding):
#   obj  [128, 800] fp8e3: rows 0:126 all 4*25200 obj logits (flat),
#                          row 126 = -pos logits, row 127 = +pos logits
#   cls  [100, 640] fp8e3: cls logits [p, a=8, c=80]
#   small[100, 72] f32:    cols 0:64 pred|gt boxes (cx,cy,w,h) [p, s, j, c],
#                          cols 64:72 host-gathered picked logits
# All tensors span >=100 partitions so every transfer engages most of the
# 16 SDMA engines; obj is split into two 64-row DMAs across the queues.
#
# On-device: softplus as exp -> ln(x*1+1) (the +1 rides the Ln activation's
# pre-bias, no elementwise pass); lse as exp -> DVE reduce -> ln with accum;
# GIoU on DVE; per-partition partials collapse to [3, 8] via a PE matmul
# with an iota-built selector so the output DMA is 3 descriptors:
#   row 0 = sums over partitions 0:126, row 1 = partition 126 (softplus(-pos)
#   sum), row 2 = partition 127 (softplus(+pos) sum); cols as listed above:
#   0 iou, 1 (enclose-union)/(enclose+eps), 2 softplus, 3 lse, 4 picked.

import numpy as np

B, N, M, C = 32, 25200, 200, 80
NCORES = 8
BPC = B // NCORES          # 4 batches per core
KP = 2                     # anchors per (partition, batch) for the cls tile
P_PAIRS = M // KP          # 100 partitions for cls-space tiles
NPAIR = BPC * KP           # 8 anchors per cls partition
P_BOX, JP = 50, 16         # box pair partitions, 16 pairs per row
P_OBJ, F_OBJ = 126, 800    # 4*25200 = 126*800 exactly
NANCH = BPC * M // P_PAIRS # 8 anchors per cls partition
W_CLS = NANCH * C          # 640
EPS = 1e-7

_CACHED_NC = None


def _emit(nc, tc, mybir, obj, cls_t, small, out):
    f32 = mybir.dt.float32
    Alu = mybir.AluOpType
    Act = mybir.ActivationFunctionType

    with tc.tile_pool(name="main", bufs=1) as pool:
        ACC = pool.tile([128, 8], f32, name="ACC")
        nc.vector.memset(ACC[:], 0.0)
        # Activation bias constants built in-block on DVE: the Bass preamble's
        # gpsimd const memsets gate the tile-enter dance, so registering our
        # own lets the (now-dead) preamble ones be stripped after compile.
        CB = pool.tile([128, 2], f32, name="CB")
        nc.vector.memset(CB[:, 0:1], 0.0)
        nc.vector.memset(CB[:, 1:2], 1.0)
        # Anchor: a dependency-free 1-col Exp so insert_act_table_loads places
        # the (1.3us) ACT_TABLE_LOAD here, overlapping the input DMAs, instead
        # of sandwiched between the first real activation's DMA waits.
        DUM = pool.tile([128, 1], f32, name="DUM")
        nc.scalar.activation(DUM[:], ACC[:, 7:8], Act.Exp, bias=CB[:, 0:1])

        OBJ = pool.tile([128, F_OBJ], mybir.dt.float8e3, name="OBJ")
        CLS = pool.tile([P_PAIRS, W_CLS], mybir.dt.float8e3, name="CLS")
        SM = pool.tile([P_BOX, 576], mybir.dt.float8e3, name="SM")
        # cls halves lead both rings (cls gates the Scalar chain and lands
        # ~8.9us), boxes ride the sync ring mid-stream, and the ring finals
        # stay the proven 64x800B obj halves (other final shapes risk a
        # stranded last sem tick). Ring depths 164 / 114 descriptors.
        nc.sync.dma_start(out=CLS[0:50], in_=cls_t.ap()[0:50])
        nc.scalar.dma_start(out=CLS[50:100], in_=cls_t.ap()[50:100])
        nc.sync.dma_start(out=SM[:], in_=small.ap())
        nc.sync.dma_start(out=OBJ[0:64], in_=obj.ap()[0:64])
        nc.scalar.dma_start(out=OBJ[64:128], in_=obj.ap()[64:128])

        # ---------------- objectness softplus ----------------
        # sum softplus(x): exp on ACT, then Ln with pre-activation bias=1.0
        # (out = ln(in*1 + 1)) with accum_out — no elementwise +1 pass needed.
        Eo = pool.tile([128, F_OBJ], f32, name="Eo")
        Lg = pool.tile([128, F_OBJ], f32, name="Lg")
        nc.scalar.activation(Eo[:], OBJ[:], Act.Exp, bias=CB[:, 0:1])

        # ---------------- classification ----------------
        Ec = pool.tile([P_PAIRS, NANCH, C], mybir.dt.bfloat16, name="Ec")
        sums = pool.tile([P_PAIRS, NANCH], f32, name="sums")
        lse = pool.tile([P_PAIRS, NANCH], f32, name="lse")
        nc.scalar.activation(
            Ec[:].rearrange("p a c -> p (a c)"), CLS[:], Act.Exp,
            bias=CB[0:P_PAIRS, 0:1],
        )
        nc.scalar.activation(Lg[:], Eo[:], Act.Ln, bias=CB[:, 1:2],
                             accum_out=ACC[0:128, 2:3])

        # ---------------- bbox GIoU term (10-op DVE chain) ----------------
        # Host ships MX = max(pred_corners, gt_corners), MN = min(...), and
        # asum+eps = areaP+areaT+eps, computed in f32 exactly as the
        # reference's _corners path. The cls reduce is emitted before the
        # chain so it runs as soon as Ec lands, off the critical path.
        nc.vector.reduce_sum(out=sums[:], in_=Ec[:], axis=mybir.AxisListType.X)
        MM = SM[:, 0:512].bitcast(f32).rearrange(
            "p (s j c) -> p s j c", s=2, c=4
        )  # s=0: MX, s=1: MN; c = (x1, y1, x2, y2)
        ASU = SM[:, 512:576].bitcast(f32)                      # asum + eps
        DE = pool.tile([P_BOX, 2, JP, 2], f32, name="DE")      # [ID || ED]
        nc.vector.tensor_sub(DE[:, 0], MM[:, 1, :, 2:4], MM[:, 0, :, 0:2])
        nc.vector.tensor_sub(DE[:, 1], MM[:, 0, :, 2:4], MM[:, 1, :, 0:2])
        DEr = pool.tile([P_BOX, 2, JP, 2], f32, name="DEr")
        nc.vector.tensor_relu(DEr[:], DE[:])   # ED >= 0 already; ID clipped
        P = pool.tile([P_BOX, 2, JP], f32, name="P")           # inter||encl
        nc.vector.tensor_mul(P[:], DEr[:, :, :, 0], DEr[:, :, :, 1])
        R = pool.tile([P_BOX, 2, JP], f32, name="R")
        R2 = pool.tile([P_BOX, 2, JP], f32, name="R2")
        nc.vector.tensor_sub(R[:, 0], ASU, P[:, 0])            # union+eps
        nc.vector.tensor_scalar_add(R[:, 1], P[:, 1], EPS)     # enclose+eps
        nc.vector.reciprocal(R2[:], R[:])
        # NOTE: tensor_tensor_reduce wedges the device (NRT_EXEC_UNIT_UNRECOVERABLE)
        # on this runtime; scalar_tensor_tensor's accum_out path works.
        t8a = pool.tile([P_BOX, JP], f32, name="t8a")
        nc.vector.scalar_tensor_tensor(
            t8a[:], P[:, 0], 1.0, R2[:, 0], Alu.mult, Alu.mult,
            accum_out=ACC[0:P_BOX, 0:1],
        )
        EmU = pool.tile([P_BOX, JP], f32, name="EmU")
        nc.vector.tensor_sub(EmU[:], R[:, 1], R[:, 0])
        t8b = pool.tile([P_BOX, JP], f32, name="t8b")
        nc.vector.scalar_tensor_tensor(
            t8b[:], EmU[:], 1.0, R2[:, 1], Alu.mult, Alu.mult,
            accum_out=ACC[0:P_BOX, 1:2],
        )

        # cls tail: Ln on ACT over the (already-running) DVE reduce output
        nc.scalar.activation(lse[:], sums[:], Act.Ln,
                             bias=CB[0:P_PAIRS, 0:1],
                             accum_out=ACC[0:P_PAIRS, 3:4])

        # Collapse ACC [128,8] to [3,8] on the idle PE so the output DMA is
        # 3 descriptors instead of 128 (cuts its HBM receipt latency, which
        # sits on the critical path into the NEFF epilogue barrier):
        # row 0 = sum over partitions 0:126, row 1 = partition 126 (-pos
        # softplus), row 2 = partition 127 (+pos softplus).
        W = pool.tile([128, 3], f32, name="W")
        IOT = pool.tile([128, 1], mybir.dt.int32, name="IOT")
        nc.gpsimd.iota(IOT[:], pattern=[[0, 1]], base=0, channel_multiplier=1)
        nc.vector.tensor_scalar(W[:, 0:1], IOT[:], P_OBJ, None,
                                op0=Alu.is_lt)
        nc.vector.tensor_scalar(W[:, 1:2], IOT[:], P_OBJ, None,
                                op0=Alu.is_equal)
        nc.vector.tensor_scalar(W[:, 2:3], IOT[:], P_OBJ + 1, None,
                                op0=Alu.is_equal)
        from concourse.bass import MemorySpace
        PS, _ps_free = tc.tile([3, 8], f32, space=MemorySpace.PSUM, name="PS")
        nc.tensor.matmul(PS[:], lhsT=W[:], rhs=ACC[:], start=True, stop=True)
        FIN = pool.tile([3, 8], f32, name="FIN")
        nc.vector.tensor_copy(FIN[:], PS[:])

        nc.sync.dma_start(out=out.ap(), in_=FIN[:])



def build_bass():
    global _CACHED_NC
    if _CACHED_NC is not None:
        return _CACHED_NC
    import concourse.bacc as bacc
    import concourse.tile as tile
    import concourse.mybir as mybir
    import concourse.bass_utils as _bu

    # The NEFF epilogue zeroes the whole sem file one EVENT_SEMAPHORE at a
    # time (~6us, serialized on the slowest engine). --max-sem-num=150 trims
    # the first few walrus-internal sems out of that clear set (3..6).
    if not hasattr(_bu, "_orig_get_walrus_args"):
        _bu._orig_get_walrus_args = _bu.get_walrus_args

        def _patched_walrus_args(*a, **k):
            return [*_bu._orig_get_walrus_args(*a, **k), "--max-sem-num=150"]

        _bu.get_walrus_args = _patched_walrus_args

    f32 = mybir.dt.float32
    bf16 = mybir.dt.bfloat16
    fp8 = mybir.dt.float8e3
    Act = mybir.ActivationFunctionType

    class FastTileContext(tile.TileContext):
        # TileContext._drain_and_barrier minus the end-of-kernel barrier and
        # the semaphore range-clear — the NEFF's own epilogue already runs a
        # pre-storm all-engine barrier, and with every kernel sem steered
        # into 207..255 the storm's Sync-engine portion (which runs after
        # Sync's drain, i.e. after the output DMA lands) covers the reset.
        def _drain_and_barrier(self, tick_clock, wait_clock):
            clock = tile.ScopedClock({None: tick_clock.global_clock})
            drain_inst = self.nc.sync.drain()
            wait_clock.add_sem_waits(drain_inst.ins, clock)
            popped = self.nc._tile_sem_poison_stack.pop()
            assert popped is self._sem_poison
            # No barrier and no explicit sem clear: the NEFF's own epilogue
            # runs a pre-storm all-engine barrier, and its Sync-engine clear
            # range (207..255) covers every sem this kernel uses.

    nc = bacc.Bacc("TRN2", target_bir_lowering=False, debug=False,
                   num_devices=NCORES)
    # Steer every tile/DMA semaphore into 207..255: keeps the live sems out
    # of the clear ranges the NEFF epilogue hands to the non-Sync engines.
    nc._state.reset_free_semaphores(list(range(207, 256)))
    obj = nc.dram_tensor("obj", [128, F_OBJ], fp8, kind="ExternalInput")
    cls_t = nc.dram_tensor("cls", [P_PAIRS, W_CLS], fp8, kind="ExternalInput")
    small = nc.dram_tensor("small", [P_BOX, 576], fp8, kind="ExternalInput")
    out = nc.dram_tensor("partials", [3, 8], f32, kind="ExternalOutput")
    with FastTileContext(nc) as tc:
        _emit(nc, tc, mybir, obj, cls_t, small, out)

    # Route every Exp/Ln to the one table that holds both, so the kernel pays
    # a single ACT_TABLE_LOAD instead of ping-ponging between per-func tables.
    orig_tables = bacc.get_activation_tables

    def _merged_tables(arch):
        out_d = {}
        for name, s in orig_tables(arch).items():
            s2 = set(s)
            if name != "natural_log_exp_and_others":
                s2.discard(Act.Exp)
                s2.discard(Act.Ln)
            out_d[name] = s2
        return out_d

    bacc.get_activation_tables = _merged_tables
    try:
        nc.compile()
    finally:
        bacc.get_activation_tables = orig_tables

    # Drop the two dead const memsets (bf16 1.0, uint8 127 — this kernel
    # never reads them): the gpsimd const chain gates the tile-enter dance,
    # so each dead memset costs ~0.1us of every core's prologue.
    entry = nc.main_func.blocks[0]
    dead_consts = ("const-bfloat16-1.0", "const-uint8-127",
                   "const-float32-0.0", "const-float32-1.0")
    entry.instructions[:] = [
        ins for ins in entry.instructions
        if not (type(ins).__name__ == "InstMemset"
                and getattr(ins, "outs", None)
                and any(d in str(ins.outs[0]) for d in dead_consts))
    ]

    # Drop a spurious default-table InstLoadActFuncSet: when two loads appear
    # with no activation between them, the first is dead and its 1.3us sits
    # right before the first Exp on the critical path.
    for blk in nc.main_func.blocks:
        loads = []
        acts_seen = set()
        for idx, ins in enumerate(blk.instructions):
            tn = type(ins).__name__
            if tn == "InstLoadActFuncSet":
                loads.append((idx, ins))
            elif tn == "InstActivation":
                acts_seen.add(len(loads))
        if len(loads) == 2 and 1 not in acts_seen and loads[0][1].sync_info is None:
            blk.instructions.pop(loads[0][0])

    _CACHED_NC = nc
    return nc


def make_in_maps(pred_bbox, pred_obj, pred_cls, gt_boxes, gt_labels):
    import ml_dtypes

    fp8 = ml_dtypes.float8_e3m4
    labels = np.asarray(gt_labels).astype(np.int64)
    in_maps = []
    picked_sums = []
    for core in range(NCORES):
        bs = slice(core * BPC, (core + 1) * BPC)

        po = np.asarray(pred_obj[bs], np.float32)
        obj = np.empty((128, F_OBJ), np.float32)
        obj[0:P_OBJ] = po.reshape(P_OBJ, F_OBJ)
        obj[P_OBJ] = -po[:, :M].reshape(F_OBJ)
        obj[P_OBJ + 1] = po[:, :M].reshape(F_OBJ)

        cl = np.asarray(pred_cls[bs, :M], np.float32).reshape(P_PAIRS, W_CLS)

        # Corner prep in f32, bit-matching the reference's _corners math.
        half = np.float32(0.5)

        def corners(b):
            r = np.asarray(b, np.float32).reshape(BPC, P_BOX, 4, 4)
            r = r.transpose(1, 0, 2, 3).reshape(P_BOX, JP, 4)
            c = np.empty((P_BOX, JP, 4), np.float32)
            c[..., 0:2] = r[..., 0:2] - r[..., 2:4] * half
            c[..., 2:4] = r[..., 0:2] + r[..., 2:4] * half
            return c

        pc = corners(pred_bbox[bs, :M])
        tc_ = corners(gt_boxes[bs])
        area = ((pc[..., 2] - pc[..., 0]) * (pc[..., 3] - pc[..., 1])
                + (tc_[..., 2] - tc_[..., 0]) * (tc_[..., 3] - tc_[..., 1]))
        sm = np.empty((P_BOX, 576), np.uint8)
        mm = np.empty((P_BOX, 2, JP, 4), np.float32)
        mm[:, 0] = np.maximum(pc, tc_)
        mm[:, 1] = np.minimum(pc, tc_)
        sm[:, 0:512] = mm.reshape(P_BOX, 128).view(np.uint8)
        sm[:, 512:576] = (area + np.float32(EPS)).reshape(P_BOX, JP).view(np.uint8)

        picked = np.take_along_axis(
            np.asarray(pred_cls[bs, :M], np.float32),
            labels[bs][..., None].astype(np.int64), axis=-1,
        )[..., 0]
        picked_sums.append(float(np.sum(picked.astype(np.float64))))

        in_maps.append({
            "obj": obj.astype(fp8),
            "cls": cl.astype(fp8),
            "small": sm.view(fp8),
        })
    return in_maps, picked_sums


def finalize(per_core_partials, picked_sums):
    s_iou = s_ratio = s_all = s_pos = s_posplus = s_lse = s_picked = 0.0
    for p, t_picked in zip(per_core_partials, picked_sums):
        p = p.astype(np.float64)
        s_iou += p[0, 0]
        s_ratio += p[0, 1]
        s_all += p[0, 2]
        s_pos += p[1, 2]
        s_posplus += p[2, 2]
        s_lse += p[0, 3]
        s_picked += t_picked
    n_pos = B * M
    n_neg = B * (N - M)
    loss_bbox = 5.0 * (n_pos - s_iou + s_ratio) / n_pos
    loss_obj = s_pos / n_pos + 0.5 * (s_all - s_posplus) / n_neg
    loss_cls = (s_lse - s_picked) / n_pos
    total = loss_bbox + loss_obj + loss_cls
    return np.array([total, loss_bbox, loss_obj, loss_cls], dtype=np.float32)


def kernel(pred_bbox, pred_obj, pred_cls, gt_boxes, gt_labels):
    from concourse.bass_utils import run_bass_kernel_spmd

    nc = build_bass()
    in_maps, picked_sums = make_in_maps(pred_bbox, pred_obj, pred_cls,
                                        gt_boxes, gt_labels)
    res = run_bass_kernel_spmd(nc, in_maps, core_ids=list(range(NCORES)))
    return finalize([r["partials"] for r in res.results], picked_sums)



# revision 15
# speedup vs baseline: 1.0807x; 1.0807x over previous
# Trainium2 Bass kernel for nn_DetectionLoss (B=32, N=25200, M=200, C=80).
#
# Strategy: pure data-parallel over batch (4 batches per core, 8 cores).
# The reference only reads pred_bbox[:, :M] and pred_cls[:, :M], so only
# those slices are shipped to the device. Each core computes per-partition
# partial sums of the loss terms; the host does the final (tiny) cross-core
# reduction and mean/lambda arithmetic in float64.
#
# Device inputs per core (host-packed; fp8e3 keeps ~1% per-element error
# that averages out over the 6400..100800-element means; boxes stay f32
# because the near-zero enclose/union denominators amplify input rounding):
#   obj  [128, 800] fp8e3: rows 0:126 all 4*25200 obj logits (flat),
#                          row 126 = -pos logits, row 127 = +pos logits
#   cls  [100, 640] fp8e3: cls logits [p, a=8, c=80]
#   small[100, 72] f32:    cols 0:64 pred|gt boxes (cx,cy,w,h) [p, s, j, c],
#                          cols 64:72 host-gathered picked logits
# All tensors span >=100 partitions so every transfer engages most of the
# 16 SDMA engines; obj is split into two 64-row DMAs across the queues.
#
# On-device: softplus as exp -> ln(x*1+1) (the +1 rides the Ln activation's
# pre-bias, no elementwise pass); lse as exp -> DVE reduce -> ln with accum;
# GIoU on DVE; per-partition partials collapse to [3, 8] via a PE matmul
# with an iota-built selector so the output DMA is 3 descriptors:
#   row 0 = sums over partitions 0:126, row 1 = partition 126 (softplus(-pos)
#   sum), row 2 = partition 127 (softplus(+pos) sum); cols as listed above:
#   0 iou, 1 (enclose-union)/(enclose+eps), 2 softplus, 3 lse, 4 picked.

import numpy as np

B, N, M, C = 32, 25200, 200, 80
NCORES = 8
BPC = B // NCORES          # 4 batches per core
KP = 2                     # anchors per (partition, batch) for the cls tile
P_PAIRS = M // KP          # 100 partitions for cls-space tiles
NPAIR = BPC * KP           # 8 anchors per cls partition
P_BOX, JP = 50, 16         # box pair partitions, 16 pairs per row
P_OBJ, F_OBJ = 126, 800    # 4*25200 = 126*800 exactly
NANCH = BPC * M // P_PAIRS # 8 anchors per cls partition
W_CLS = NANCH * C          # 640
EPS = 1e-7

_CACHED_NC = None


def _emit(nc, tc, mybir, obj, cls_t, small, out):
    f32 = mybir.dt.float32
    Alu = mybir.AluOpType
    Act = mybir.ActivationFunctionType

    with tc.tile_pool(name="main", bufs=1) as pool:
        ACC = pool.tile([128, 8], f32, name="ACC")
        nc.vector.memset(ACC[:], 0.0)
        # Activation bias constants built in-block on DVE: the Bass preamble's
        # gpsimd const memsets gate the tile-enter dance, so registering our
        # own lets the (now-dead) preamble ones be stripped after compile.
        CB = pool.tile([128, 2], f32, name="CB")
        nc.vector.memset(CB[:, 0:1], 0.0)
        nc.vector.memset(CB[:, 1:2], 1.0)
        # Anchor: a dependency-free 1-col Exp so insert_act_table_loads places
        # the (1.3us) ACT_TABLE_LOAD here, overlapping the input DMAs, instead
        # of sandwiched between the first real activation's DMA waits.
        DUM = pool.tile([128, 1], f32, name="DUM")
        nc.scalar.activation(DUM[:], ACC[:, 7:8], Act.Exp, bias=CB[:, 0:1])

        OBJ = pool.tile([128, F_OBJ], mybir.dt.float8e3, name="OBJ")
        CLS = pool.tile([P_PAIRS, W_CLS], mybir.dt.float8e3, name="CLS")
        SM = pool.tile([P_BOX, 576], mybir.dt.float8e3, name="SM")
        # cls halves lead both rings (cls gates the Scalar chain and lands
        # ~8.9us), box halves ride both rings mid-stream, and the ring
        # finals stay the proven 64x800B obj halves (other final shapes risk
        # a stranded last sem tick). Balanced ring depths: 139 / 139.
        nc.sync.dma_start(out=CLS[0:50], in_=cls_t.ap()[0:50])
        nc.scalar.dma_start(out=CLS[50:100], in_=cls_t.ap()[50:100])
        nc.sync.dma_start(out=SM[0:25], in_=small.ap()[0:25])
        nc.scalar.dma_start(out=SM[25:50], in_=small.ap()[25:50])
        nc.sync.dma_start(out=OBJ[0:64], in_=obj.ap()[0:64])
        nc.scalar.dma_start(out=OBJ[64:128], in_=obj.ap()[64:128])

        # ---------------- objectness softplus ----------------
        # sum softplus(x): exp on ACT, then Ln with pre-activation bias=1.0
        # (out = ln(in*1 + 1)) with accum_out — no elementwise +1 pass needed.
        Eo = pool.tile([128, F_OBJ], f32, name="Eo")
        Lg = pool.tile([128, F_OBJ], f32, name="Lg")
        nc.scalar.activation(Eo[:], OBJ[:], Act.Exp, bias=CB[:, 0:1])

        # ---------------- classification ----------------
        Ec = pool.tile([P_PAIRS, NANCH, C], mybir.dt.bfloat16, name="Ec")
        sums = pool.tile([P_PAIRS, NANCH], f32, name="sums")
        lse = pool.tile([P_PAIRS, NANCH], f32, name="lse")
        nc.scalar.activation(
            Ec[:].rearrange("p a c -> p (a c)"), CLS[:], Act.Exp,
            bias=CB[0:P_PAIRS, 0:1],
        )
        nc.scalar.activation(Lg[:], Eo[:], Act.Ln, bias=CB[:, 1:2],
                             accum_out=ACC[0:128, 2:3])

        # ---------------- bbox GIoU term (10-op DVE chain) ----------------
        # Host ships MX = max(pred_corners, gt_corners), MN = min(...), and
        # asum+eps = areaP+areaT+eps, computed in f32 exactly as the
        # reference's _corners path. The cls reduce is emitted before the
        # chain so it runs as soon as Ec lands, off the critical path.
        nc.vector.reduce_sum(out=sums[:], in_=Ec[:], axis=mybir.AxisListType.X)
        MM = SM[:, 0:512].bitcast(f32).rearrange(
            "p (s j c) -> p s j c", s=2, c=4
        )  # s=0: MX, s=1: MN; c = (x1, y1, x2, y2)
        ASU = SM[:, 512:576].bitcast(f32)                      # asum + eps
        DE = pool.tile([P_BOX, 2, JP, 2], f32, name="DE")      # [ID || ED]
        nc.vector.tensor_sub(DE[:, 0], MM[:, 1, :, 2:4], MM[:, 0, :, 0:2])
        nc.vector.tensor_sub(DE[:, 1], MM[:, 0, :, 2:4], MM[:, 1, :, 0:2])
        DEr = pool.tile([P_BOX, 2, JP, 2], f32, name="DEr")
        nc.vector.tensor_relu(DEr[:], DE[:])   # ED >= 0 already; ID clipped
        P = pool.tile([P_BOX, 2, JP], f32, name="P")           # inter||encl
        nc.vector.tensor_mul(P[:], DEr[:, :, :, 0], DEr[:, :, :, 1])
        R = pool.tile([P_BOX, 2, JP], f32, name="R")
        R2 = pool.tile([P_BOX, 2, JP], f32, name="R2")
        nc.vector.tensor_sub(R[:, 0], ASU, P[:, 0])            # union+eps
        nc.vector.tensor_scalar_add(R[:, 1], P[:, 1], EPS)     # enclose+eps
        nc.vector.reciprocal(R2[:], R[:])
        # NOTE: tensor_tensor_reduce wedges the device (NRT_EXEC_UNIT_UNRECOVERABLE)
        # on this runtime; scalar_tensor_tensor's accum_out path works.
        t8a = pool.tile([P_BOX, JP], f32, name="t8a")
        nc.vector.scalar_tensor_tensor(
            t8a[:], P[:, 0], 1.0, R2[:, 0], Alu.mult, Alu.mult,
            accum_out=ACC[0:P_BOX, 0:1],
        )
        EmU = pool.tile([P_BOX, JP], f32, name="EmU")
        nc.vector.tensor_sub(EmU[:], R[:, 1], R[:, 0])
        t8b = pool.tile([P_BOX, JP], f32, name="t8b")
        nc.vector.scalar_tensor_tensor(
            t8b[:], EmU[:], 1.0, R2[:, 1], Alu.mult, Alu.mult,
            accum_out=ACC[0:P_BOX, 1:2],
        )

        # cls tail: Ln on ACT over the (already-running) DVE reduce output
        nc.scalar.activation(lse[:], sums[:], Act.Ln,
                             bias=CB[0:P_PAIRS, 0:1],
                             accum_out=ACC[0:P_PAIRS, 3:4])

        # Collapse ACC [128,8] to [3,8] on the idle PE so the output DMA is
        # 3 descriptors instead of 128 (cuts its HBM receipt latency, which
        # sits on the critical path into the NEFF epilogue barrier):
        # row 0 = sum over partitions 0:126, row 1 = partition 126 (-pos
        # softplus), row 2 = partition 127 (+pos softplus).
        W = pool.tile([128, 3], f32, name="W")
        IOT = pool.tile([128, 1], mybir.dt.int32, name="IOT")
        nc.gpsimd.iota(IOT[:], pattern=[[0, 1]], base=0, channel_multiplier=1)
        nc.vector.tensor_scalar(W[:, 0:1], IOT[:], P_OBJ, None,
                                op0=Alu.is_lt)
        nc.vector.tensor_scalar(W[:, 1:2], IOT[:], P_OBJ, None,
                                op0=Alu.is_equal)
        nc.vector.tensor_scalar(W[:, 2:3], IOT[:], P_OBJ + 1, None,
                                op0=Alu.is_equal)
        from concourse.bass import MemorySpace
        PS, _ps_free = tc.tile([3, 8], f32, space=MemorySpace.PSUM, name="PS")
        nc.tensor.matmul(PS[:], lhsT=W[:], rhs=ACC[:], start=True, stop=True)
        FIN = pool.tile([3, 8], f32, name="FIN")
        nc.vector.tensor_copy(FIN[:], PS[:])

        nc.sync.dma_start(out=out.ap(), in_=FIN[:])



def build_bass():
    global _CACHED_NC
    if _CACHED_NC is not None:
        return _CACHED_NC
    import concourse.bacc as bacc
    import concourse.tile as tile
    import concourse.mybir as mybir
    import concourse.bass_utils as _bu

    # The NEFF epilogue zeroes the whole sem file one EVENT_SEMAPHORE at a
    # time (~6us, serialized on the slowest engine). --max-sem-num=150 trims
    # the first few walrus-internal sems out of that clear set (3..6).
    if not hasattr(_bu, "_orig_get_walrus_args"):
        _bu._orig_get_walrus_args = _bu.get_walrus_args

        def _patched_walrus_args(*a, **k):
            return [*_bu._orig_get_walrus_args(*a, **k), "--max-sem-num=150"]

        _bu.get_walrus_args = _patched_walrus_args

    f32 = mybir.dt.float32
    bf16 = mybir.dt.bfloat16
    fp8 = mybir.dt.float8e3
    Act = mybir.ActivationFunctionType

    class FastTileContext(tile.TileContext):
        # TileContext._drain_and_barrier minus the end-of-kernel barrier and
        # the semaphore range-clear — the NEFF's own epilogue already runs a
        # pre-storm all-engine barrier, and with every kernel sem steered
        # into 207..255 the storm's Sync-engine portion (which runs after
        # Sync's drain, i.e. after the output DMA lands) covers the reset.
        def _drain_and_barrier(self, tick_clock, wait_clock):
            clock = tile.ScopedClock({None: tick_clock.global_clock})
            drain_inst = self.nc.sync.drain()
            wait_clock.add_sem_waits(drain_inst.ins, clock)
            popped = self.nc._tile_sem_poison_stack.pop()
            assert popped is self._sem_poison
            # No barrier and no explicit sem clear: the NEFF's own epilogue
            # runs a pre-storm all-engine barrier, and its Sync-engine clear
            # range (207..255) covers every sem this kernel uses.

    nc = bacc.Bacc("TRN2", target_bir_lowering=False, debug=False,
                   num_devices=NCORES)
    # Steer every tile/DMA semaphore into 207..255: keeps the live sems out
    # of the clear ranges the NEFF epilogue hands to the non-Sync engines.
    nc._state.reset_free_semaphores(list(range(207, 256)))
    obj = nc.dram_tensor("obj", [128, F_OBJ], fp8, kind="ExternalInput")
    cls_t = nc.dram_tensor("cls", [P_PAIRS, W_CLS], fp8, kind="ExternalInput")
    small = nc.dram_tensor("small", [P_BOX, 576], fp8, kind="ExternalInput")
    out = nc.dram_tensor("partials", [3, 8], f32, kind="ExternalOutput")
    with FastTileContext(nc) as tc:
        _emit(nc, tc, mybir, obj, cls_t, small, out)

    # Route every Exp/Ln to the one table that holds both, so the kernel pays
    # a single ACT_TABLE_LOAD instead of ping-ponging between per-func tables.
    orig_tables = bacc.get_activation_tables

    def _merged_tables(arch):
        out_d = {}
        for name, s in orig_tables(arch).items():
            s2 = set(s)
            if name != "natural_log_exp_and_others":
                s2.discard(Act.Exp)
                s2.discard(Act.Ln)
            out_d[name] = s2
        return out_d

    bacc.get_activation_tables = _merged_tables
    try:
        nc.compile()
    finally:
        bacc.get_activation_tables = orig_tables

    # Drop the two dead const memsets (bf16 1.0, uint8 127 — this kernel
    # never reads them): the gpsimd const chain gates the tile-enter dance,
    # so each dead memset costs ~0.1us of every core's prologue.
    entry = nc.main_func.blocks[0]
    dead_consts = ("const-bfloat16-1.0", "const-uint8-127",
                   "const-float32-0.0", "const-float32-1.0")
    entry.instructions[:] = [
        ins for ins in entry.instructions
        if not (type(ins).__name__ == "InstMemset"
                and getattr(ins, "outs", None)
                and any(d in str(ins.outs[0]) for d in dead_consts))
    ]

    # Drop a spurious default-table InstLoadActFuncSet: when two loads appear
    # with no activation between them, the first is dead and its 1.3us sits
    # right before the first Exp on the critical path.
    for blk in nc.main_func.blocks:
        loads = []
        acts_seen = set()
        for idx, ins in enumerate(blk.instructions):
            tn = type(ins).__name__
            if tn == "InstLoadActFuncSet":
                loads.append((idx, ins))
            elif tn == "InstActivation":
                acts_seen.add(len(loads))
        if len(loads) == 2 and 1 not in acts_seen and loads[0][1].sync_info is None:
            blk.instructions.pop(loads[0][0])

    _CACHED_NC = nc
    return nc


def make_in_maps(pred_bbox, pred_obj, pred_cls, gt_boxes, gt_labels):
    import ml_dtypes

    fp8 = ml_dtypes.float8_e3m4
    labels = np.asarray(gt_labels).astype(np.int64)
    in_maps = []
    picked_sums = []
    for core in range(NCORES):
        bs = slice(core * BPC, (core + 1) * BPC)

        po = np.asarray(pred_obj[bs], np.float32)
        obj = np.empty((128, F_OBJ), np.float32)
        obj[0:P_OBJ] = po.reshape(P_OBJ, F_OBJ)
        obj[P_OBJ] = -po[:, :M].reshape(F_OBJ)
        obj[P_OBJ + 1] = po[:, :M].reshape(F_OBJ)

        cl = np.asarray(pred_cls[bs, :M], np.float32).reshape(P_PAIRS, W_CLS)

        # Corner prep in f32, bit-matching the reference's _corners math.
        half = np.float32(0.5)

        def corners(b):
            r = np.asarray(b, np.float32).reshape(BPC, P_BOX, 4, 4)
            r = r.transpose(1, 0, 2, 3).reshape(P_BOX, JP, 4)
            c = np.empty((P_BOX, JP, 4), np.float32)
            c[..., 0:2] = r[..., 0:2] - r[..., 2:4] * half
            c[..., 2:4] = r[..., 0:2] + r[..., 2:4] * half
            return c

        pc = corners(pred_bbox[bs, :M])
        tc_ = corners(gt_boxes[bs])
        area = ((pc[..., 2] - pc[..., 0]) * (pc[..., 3] - pc[..., 1])
                + (tc_[..., 2] - tc_[..., 0]) * (tc_[..., 3] - tc_[..., 1]))
        sm = np.empty((P_BOX, 576), np.uint8)
        mm = np.empty((P_BOX, 2, JP, 4), np.float32)
        mm[:, 0] = np.maximum(pc, tc_)
        mm[:, 1] = np.minimum(pc, tc_)
        sm[:, 0:512] = mm.reshape(P_BOX, 128).view(np.uint8)
        sm[:, 512:576] = (area + np.float32(EPS)).reshape(P_BOX, JP).view(np.uint8)

        picked = np.take_along_axis(
            np.asarray(pred_cls[bs, :M], np.float32),
            labels[bs][..., None].astype(np.int64), axis=-1,
        )[..., 0]
        picked_sums.append(float(np.sum(picked.astype(np.float64))))

        in_maps.append({
            "obj": obj.astype(fp8),
            "cls": cl.astype(fp8),
            "small": sm.view(fp8),
        })
    return in_maps, picked_sums


def finalize(per_core_partials, picked_sums):
    s_iou = s_ratio = s_all = s_pos = s_posplus = s_lse = s_picked = 0.0
    for p, t_picked in zip(per_core_partials, picked_sums):
        p = p.astype(np.float64)
        s_iou += p[0, 0]
        s_ratio += p[0, 1]
        s_all += p[0, 2]
        s_pos += p[1, 2]
        s_posplus += p[2, 2]
        s_lse += p[0, 3]
        s_picked += t_picked
    n_pos = B * M
    n_neg = B * (N - M)
    loss_bbox = 5.0 * (n_pos - s_iou + s_ratio) / n_pos
    loss_obj = s_pos / n_pos + 0.5 * (s_all - s_posplus) / n_neg
    loss_cls = (s_lse - s_picked) / n_pos
    total = loss_bbox + loss_obj + loss_cls
    return np.array([total, loss_bbox, loss_obj, loss_cls], dtype=np.float32)


def kernel(pred_bbox, pred_obj, pred_cls, gt_boxes, gt_labels):
    from concourse.bass_utils import run_bass_kernel_spmd

    nc = build_bass()
    in_maps, picked_sums = make_in_maps(pred_bbox, pred_obj, pred_cls,
                                        gt_boxes, gt_labels)
    res = run_bass_kernel_spmd(nc, in_maps, core_ids=list(range(NCORES)))
    return finalize([r["partials"] for r in res.results], picked_sums)



# revision 16
# speedup vs baseline: 1.1198x; 1.0363x over previous
# Trainium2 Bass kernel for nn_DetectionLoss (B=32, N=25200, M=200, C=80).
#
# Strategy: pure data-parallel over batch (4 batches per core, 8 cores).
# The reference only reads pred_bbox[:, :M] and pred_cls[:, :M], so only
# those slices are shipped to the device. Each core computes per-partition
# partial sums of the loss terms; the host does the final (tiny) cross-core
# reduction and mean/lambda arithmetic in float64.
#
# Device inputs per core (host-packed; fp8e3 keeps ~1% per-element error
# that averages out over the 6400..100800-element means; boxes stay f32
# because the near-zero enclose/union denominators amplify input rounding):
#   obj  [128, 800] fp8e3: rows 0:126 all 4*25200 obj logits (flat),
#                          row 126 = -pos logits, row 127 = +pos logits
#   cls  [100, 640] fp8e3: cls logits [p, a=8, c=80]
#   small[100, 72] f32:    cols 0:64 pred|gt boxes (cx,cy,w,h) [p, s, j, c],
#                          cols 64:72 host-gathered picked logits
# All tensors span >=100 partitions so every transfer engages most of the
# 16 SDMA engines; obj is split into two 64-row DMAs across the queues.
#
# On-device: softplus as exp -> ln(x*1+1) (the +1 rides the Ln activation's
# pre-bias, no elementwise pass); lse as exp -> DVE reduce -> ln with accum;
# GIoU on DVE; per-partition partials collapse to [3, 8] via a PE matmul
# with an iota-built selector so the output DMA is 3 descriptors:
#   row 0 = sums over partitions 0:126, row 1 = partition 126 (softplus(-pos)
#   sum), row 2 = partition 127 (softplus(+pos) sum); cols as listed above:
#   0 iou, 1 (enclose-union)/(enclose+eps), 2 softplus, 3 lse, 4 picked.

import numpy as np

B, N, M, C = 32, 25200, 200, 80
NCORES = 8
BPC = B // NCORES          # 4 batches per core
KP = 2                     # anchors per (partition, batch) for the cls tile
P_PAIRS = M // KP          # 100 partitions for cls-space tiles
NPAIR = BPC * KP           # 8 anchors per cls partition
P_BOX, JP = 50, 16         # box pair partitions, 16 pairs per row
P_OBJ, F_OBJ = 126, 800    # 4*25200 = 126*800 exactly
NANCH = BPC * M // P_PAIRS # 8 anchors per cls partition
W_CLS = NANCH * C          # 640
EPS = 1e-7

_CACHED_NC = None


def _emit(nc, tc, mybir, obj, cls_t, small, out):
    f32 = mybir.dt.float32
    Alu = mybir.AluOpType
    Act = mybir.ActivationFunctionType

    with tc.tile_pool(name="main", bufs=1) as pool:
        ACC = pool.tile([128, 8], f32, name="ACC")
        nc.vector.memset(ACC[:], 0.0)
        # Activation bias constants built in-block on DVE: the Bass preamble's
        # gpsimd const memsets gate the tile-enter dance, so registering our
        # own lets the (now-dead) preamble ones be stripped after compile.
        CB = pool.tile([128, 2], f32, name="CB")
        nc.vector.memset(CB[:, 0:1], 0.0)
        nc.vector.memset(CB[:, 1:2], 1.0)
        # Anchor: a dependency-free 1-col Exp so insert_act_table_loads places
        # the (1.3us) ACT_TABLE_LOAD here, overlapping the input DMAs, instead
        # of sandwiched between the first real activation's DMA waits.
        DUM = pool.tile([128, 1], f32, name="DUM")
        nc.scalar.activation(DUM[:], ACC[:, 7:8], Act.Exp, bias=CB[:, 0:1])

        OBJ = pool.tile([128, F_OBJ], mybir.dt.float8e3, name="OBJ")
        CLS = pool.tile([P_PAIRS, W_CLS], mybir.dt.float8e3, name="CLS")
        SM = pool.tile([P_BOX, 576], mybir.dt.float8e3, name="SM")
        # cls halves lead both rings (cls gates the Scalar chain and lands
        # ~8.9us), boxes ride the sync ring mid-stream, and the ring finals
        # stay the proven 64x800B obj halves (other final shapes risk a
        # stranded last sem tick). Ring depths 164 / 114 descriptors.
        nc.sync.dma_start(out=CLS[0:50], in_=cls_t.ap()[0:50])
        nc.scalar.dma_start(out=CLS[50:100], in_=cls_t.ap()[50:100])
        nc.sync.dma_start(out=SM[:], in_=small.ap())
        nc.sync.dma_start(out=OBJ[0:64], in_=obj.ap()[0:64])
        nc.scalar.dma_start(out=OBJ[64:128], in_=obj.ap()[64:128])

        # ---------------- objectness softplus ----------------
        # sum softplus(x): exp on ACT, then Ln with pre-activation bias=1.0
        # (out = ln(in*1 + 1)) with accum_out — no elementwise +1 pass needed.
        Eo = pool.tile([128, F_OBJ], f32, name="Eo")
        Lg = pool.tile([128, F_OBJ], f32, name="Lg")
        nc.scalar.activation(Eo[:], OBJ[:], Act.Exp, bias=CB[:, 0:1])

        # ---------------- classification ----------------
        Ec = pool.tile([P_PAIRS, NANCH, C], mybir.dt.bfloat16, name="Ec")
        sums = pool.tile([P_PAIRS, NANCH], f32, name="sums")
        lse = pool.tile([P_PAIRS, NANCH], f32, name="lse")
        nc.scalar.activation(
            Ec[:].rearrange("p a c -> p (a c)"), CLS[:], Act.Exp,
            bias=CB[0:P_PAIRS, 0:1],
        )
        nc.scalar.activation(Lg[:], Eo[:], Act.Ln, bias=CB[:, 1:2],
                             accum_out=ACC[0:128, 2:3])

        # ---------------- bbox GIoU term (10-op DVE chain) ----------------
        # Host ships MX = max(pred_corners, gt_corners), MN = min(...), and
        # asum+eps = areaP+areaT+eps, computed in f32 exactly as the
        # reference's _corners path. The cls reduce is emitted before the
        # chain so it runs as soon as Ec lands, off the critical path.
        nc.vector.reduce_sum(out=sums[:], in_=Ec[:], axis=mybir.AxisListType.X)
        MM = SM[:, 0:512].bitcast(f32).rearrange(
            "p (s j c) -> p s j c", s=2, c=4
        )  # s=0: MX, s=1: MN; c = (x1, y1, x2, y2)
        ASU = SM[:, 512:576].bitcast(f32)                      # asum + eps
        DE = pool.tile([P_BOX, 2, JP, 2], f32, name="DE")      # [ID || ED]
        nc.vector.tensor_sub(DE[:, 0], MM[:, 1, :, 2:4], MM[:, 0, :, 0:2])
        nc.vector.tensor_sub(DE[:, 1], MM[:, 0, :, 2:4], MM[:, 1, :, 0:2])
        DEr = pool.tile([P_BOX, 2, JP, 2], f32, name="DEr")
        nc.vector.tensor_relu(DEr[:], DE[:])   # ED >= 0 already; ID clipped
        P = pool.tile([P_BOX, 2, JP], f32, name="P")           # inter||encl
        nc.vector.tensor_mul(P[:], DEr[:, :, :, 0], DEr[:, :, :, 1])
        R = pool.tile([P_BOX, 2, JP], f32, name="R")
        R2 = pool.tile([P_BOX, 2, JP], f32, name="R2")
        nc.vector.tensor_sub(R[:, 0], ASU, P[:, 0])            # union+eps
        nc.vector.tensor_scalar_add(R[:, 1], P[:, 1], EPS)     # enclose+eps
        nc.vector.reciprocal(R2[:], R[:])
        # NOTE: tensor_tensor_reduce wedges the device (NRT_EXEC_UNIT_UNRECOVERABLE)
        # on this runtime; scalar_tensor_tensor's accum_out path works.
        t8a = pool.tile([P_BOX, JP], f32, name="t8a")
        nc.vector.scalar_tensor_tensor(
            t8a[:], P[:, 0], 1.0, R2[:, 0], Alu.mult, Alu.mult,
            accum_out=ACC[0:P_BOX, 0:1],
        )
        EmU = pool.tile([P_BOX, JP], f32, name="EmU")
        nc.vector.tensor_sub(EmU[:], R[:, 1], R[:, 0])
        t8b = pool.tile([P_BOX, JP], f32, name="t8b")
        nc.vector.scalar_tensor_tensor(
            t8b[:], EmU[:], 1.0, R2[:, 1], Alu.mult, Alu.mult,
            accum_out=ACC[0:P_BOX, 1:2],
        )

        # cls tail: Ln on ACT over the (already-running) DVE reduce output
        nc.scalar.activation(lse[:], sums[:], Act.Ln,
                             bias=CB[0:P_PAIRS, 0:1],
                             accum_out=ACC[0:P_PAIRS, 3:4])

        # Collapse ACC [128,8] to [3,8] on the idle PE so the output DMA is
        # 3 descriptors instead of 128 (cuts its HBM receipt latency, which
        # sits on the critical path into the NEFF epilogue barrier):
        # row 0 = sum over partitions 0:126, row 1 = partition 126 (-pos
        # softplus), row 2 = partition 127 (+pos softplus).
        W = pool.tile([128, 3], f32, name="W")
        IOT = pool.tile([128, 1], mybir.dt.int32, name="IOT")
        nc.gpsimd.iota(IOT[:], pattern=[[0, 1]], base=0, channel_multiplier=1)
        nc.vector.tensor_scalar(W[:, 0:1], IOT[:], P_OBJ, None,
                                op0=Alu.is_lt)
        nc.vector.tensor_scalar(W[:, 1:2], IOT[:], P_OBJ, None,
                                op0=Alu.is_equal)
        nc.vector.tensor_scalar(W[:, 2:3], IOT[:], P_OBJ + 1, None,
                                op0=Alu.is_equal)
        from concourse.bass import MemorySpace
        PS, _ps_free = tc.tile([3, 8], f32, space=MemorySpace.PSUM, name="PS")
        nc.tensor.matmul(PS[:], lhsT=W[:], rhs=ACC[:], start=True, stop=True)
        FIN = pool.tile([3, 8], f32, name="FIN")
        nc.vector.tensor_copy(FIN[:], PS[:])

        nc.sync.dma_start(out=out.ap(), in_=FIN[:])



def build_bass():
    global _CACHED_NC
    if _CACHED_NC is not None:
        return _CACHED_NC
    import concourse.bacc as bacc
    import concourse.tile as tile
    import concourse.mybir as mybir
    import concourse.bass_utils as _bu

    # The NEFF epilogue zeroes the whole sem file one EVENT_SEMAPHORE at a
    # time (~6us, serialized on the slowest engine). --max-sem-num=150 trims
    # the first few walrus-internal sems out of that clear set (3..6).
    if not hasattr(_bu, "_orig_get_walrus_args"):
        _bu._orig_get_walrus_args = _bu.get_walrus_args

        def _patched_walrus_args(*a, **k):
            return [*_bu._orig_get_walrus_args(*a, **k), "--max-sem-num=150"]

        _bu.get_walrus_args = _patched_walrus_args

    f32 = mybir.dt.float32
    bf16 = mybir.dt.bfloat16
    fp8 = mybir.dt.float8e3
    Act = mybir.ActivationFunctionType

    class FastTileContext(tile.TileContext):
        # TileContext._drain_and_barrier minus the end-of-kernel barrier and
        # the semaphore range-clear — the NEFF's own epilogue already runs a
        # pre-storm all-engine barrier, and with every kernel sem steered
        # into 207..255 the storm's Sync-engine portion (which runs after
        # Sync's drain, i.e. after the output DMA lands) covers the reset.
        def _drain_and_barrier(self, tick_clock, wait_clock):
            clock = tile.ScopedClock({None: tick_clock.global_clock})
            drain_inst = self.nc.sync.drain()
            wait_clock.add_sem_waits(drain_inst.ins, clock)
            popped = self.nc._tile_sem_poison_stack.pop()
            assert popped is self._sem_poison
            # No barrier and no explicit sem clear: the NEFF's own epilogue
            # runs a pre-storm all-engine barrier, and its Sync-engine clear
            # range (207..255) covers every sem this kernel uses.

    nc = bacc.Bacc("TRN2", target_bir_lowering=False, debug=False,
                   num_devices=NCORES)
    # Steer every tile/DMA semaphore into 207..255: keeps the live sems out
    # of the clear ranges the NEFF epilogue hands to the non-Sync engines.
    nc._state.reset_free_semaphores(list(range(207, 256)))
    obj = nc.dram_tensor("obj", [128, F_OBJ], fp8, kind="ExternalInput")
    cls_t = nc.dram_tensor("cls", [P_PAIRS, W_CLS], fp8, kind="ExternalInput")
    small = nc.dram_tensor("small", [P_BOX, 576], fp8, kind="ExternalInput")
    out = nc.dram_tensor("partials", [3, 8], f32, kind="ExternalOutput")
    with FastTileContext(nc) as tc:
        _emit(nc, tc, mybir, obj, cls_t, small, out)

    # Route every Exp/Ln to the one table that holds both, so the kernel pays
    # a single ACT_TABLE_LOAD instead of ping-ponging between per-func tables.
    orig_tables = bacc.get_activation_tables

    def _merged_tables(arch):
        out_d = {}
        for name, s in orig_tables(arch).items():
            s2 = set(s)
            if name != "natural_log_exp_and_others":
                s2.discard(Act.Exp)
                s2.discard(Act.Ln)
            out_d[name] = s2
        return out_d

    bacc.get_activation_tables = _merged_tables
    try:
        nc.compile()
    finally:
        bacc.get_activation_tables = orig_tables

    # Drop the two dead const memsets (bf16 1.0, uint8 127 — this kernel
    # never reads them): the gpsimd const chain gates the tile-enter dance,
    # so each dead memset costs ~0.1us of every core's prologue.
    entry = nc.main_func.blocks[0]
    dead_consts = ("const-bfloat16-1.0", "const-uint8-127",
                   "const-float32-0.0", "const-float32-1.0")
    entry.instructions[:] = [
        ins for ins in entry.instructions
        if not (type(ins).__name__ == "InstMemset"
                and getattr(ins, "outs", None)
                and any(d in str(ins.outs[0]) for d in dead_consts))
    ]

    # Drop a spurious default-table InstLoadActFuncSet: when two loads appear
    # with no activation between them, the first is dead and its 1.3us sits
    # right before the first Exp on the critical path.
    for blk in nc.main_func.blocks:
        loads = []
        acts_seen = set()
        for idx, ins in enumerate(blk.instructions):
            tn = type(ins).__name__
            if tn == "InstLoadActFuncSet":
                loads.append((idx, ins))
            elif tn == "InstActivation":
                acts_seen.add(len(loads))
        if len(loads) == 2 and 1 not in acts_seen and loads[0][1].sync_info is None:
            blk.instructions.pop(loads[0][0])

    _CACHED_NC = nc
    return nc


def make_in_maps(pred_bbox, pred_obj, pred_cls, gt_boxes, gt_labels):
    import ml_dtypes

    fp8 = ml_dtypes.float8_e3m4
    labels = np.asarray(gt_labels).astype(np.int64)
    in_maps = []
    picked_sums = []
    for core in range(NCORES):
        bs = slice(core * BPC, (core + 1) * BPC)

        po = np.asarray(pred_obj[bs], np.float32)
        obj = np.empty((128, F_OBJ), np.float32)
        obj[0:P_OBJ] = po.reshape(P_OBJ, F_OBJ)
        obj[P_OBJ] = -po[:, :M].reshape(F_OBJ)
        obj[P_OBJ + 1] = po[:, :M].reshape(F_OBJ)

        cl = np.asarray(pred_cls[bs, :M], np.float32).reshape(P_PAIRS, W_CLS)

        # Corner prep in f32, bit-matching the reference's _corners math.
        half = np.float32(0.5)

        def corners(b):
            r = np.asarray(b, np.float32).reshape(BPC, P_BOX, 4, 4)
            r = r.transpose(1, 0, 2, 3).reshape(P_BOX, JP, 4)
            c = np.empty((P_BOX, JP, 4), np.float32)
            c[..., 0:2] = r[..., 0:2] - r[..., 2:4] * half
            c[..., 2:4] = r[..., 0:2] + r[..., 2:4] * half
            return c

        pc = corners(pred_bbox[bs, :M])
        tc_ = corners(gt_boxes[bs])
        area = ((pc[..., 2] - pc[..., 0]) * (pc[..., 3] - pc[..., 1])
                + (tc_[..., 2] - tc_[..., 0]) * (tc_[..., 3] - tc_[..., 1]))
        sm = np.empty((P_BOX, 576), np.uint8)
        mm = np.empty((P_BOX, 2, JP, 4), np.float32)
        mm[:, 0] = np.maximum(pc, tc_)
        mm[:, 1] = np.minimum(pc, tc_)
        sm[:, 0:512] = mm.reshape(P_BOX, 128).view(np.uint8)
        sm[:, 512:576] = (area + np.float32(EPS)).reshape(P_BOX, JP).view(np.uint8)

        picked = np.take_along_axis(
            np.asarray(pred_cls[bs, :M], np.float32),
            labels[bs][..., None].astype(np.int64), axis=-1,
        )[..., 0]
        picked_sums.append(float(np.sum(picked.astype(np.float64))))

        in_maps.append({
            "obj": obj.astype(fp8),
            "cls": cl.astype(fp8),
            "small": sm.view(fp8),
        })
    return in_maps, picked_sums


def finalize(per_core_partials, picked_sums):
    s_iou = s_ratio = s_all = s_pos = s_posplus = s_lse = s_picked = 0.0
    for p, t_picked in zip(per_core_partials, picked_sums):
        p = p.astype(np.float64)
        s_iou += p[0, 0]
        s_ratio += p[0, 1]
        s_all += p[0, 2]
        s_pos += p[1, 2]
        s_posplus += p[2, 2]
        s_lse += p[0, 3]
        s_picked += t_picked
    n_pos = B * M
    n_neg = B * (N - M)
    loss_bbox = 5.0 * (n_pos - s_iou + s_ratio) / n_pos
    loss_obj = s_pos / n_pos + 0.5 * (s_all - s_posplus) / n_neg
    loss_cls = (s_lse - s_picked) / n_pos
    total = loss_bbox + loss_obj + loss_cls
    return np.array([total, loss_bbox, loss_obj, loss_cls], dtype=np.float32)


def kernel(pred_bbox, pred_obj, pred_cls, gt_boxes, gt_labels):
    from concourse.bass_utils import run_bass_kernel_spmd

    nc = build_bass()
    in_maps, picked_sums = make_in_maps(pred_bbox, pred_obj, pred_cls,
                                        gt_boxes, gt_labels)
    res = run_bass_kernel_spmd(nc, in_maps, core_ids=list(range(NCORES)))
    return finalize([r["partials"] for r in res.results], picked_sums)

